# revision 41
# baseline (speedup 1.0000x reference)
"""Char-GRU (3-layer GRU + causal attention LM) Trainium2 kernel.

Sharding: data-parallel over batch. B=16 across 8 cores -> 2 batch elems/core.

Device layouts (per core, BL=2 local batch, token index tok = t*BL + b):
  - GRU state h:         (128, KH, BL)  partition p = h%128, k = h//128
  - xi (input gates):    (128, MJ, C*BL) per wave; m-chunk j = (3H)//128
  - wave output out_l.T: (128, KH, C*BL)
  - attention:           scores.T (kt-part, q-free); everything H-on-partition
"""

import numpy as np
import ml_dtypes

import concourse.bass as bass
import concourse.tile as tile
from concourse import bacc, mybir
from concourse.bass import ds, ts
from concourse.bass_utils import run_bass_kernel_spmd
from concourse.masks import make_identity

BF = mybir.dt.bfloat16
F32 = mybir.dt.float32
AF = mybir.ActivationFunctionType
OP = mybir.AluOpType
NPBF = ml_dtypes.bfloat16

NC = 8
B, V, E, H = 16, 72, 128, 512
BL = B // NC
H3 = 3 * H
KH = H // 128   # 4
MJ = H3 // 128  # 12

# full-size config; test.py may call _set_mode for a small smoke config
_CFG = dict(L=2048, C=512, U=8)


def _set_mode(L, C, U):
    _CFG.update(L=L, C=C, U=U)
    _BUILT.clear()


_BUILT = {}
_LAST_RES = None


def _build_program():
    L, C, U = _CFG["L"], _CFG["C"], _CFG["U"]
    NW = L // C
    TOK = BL * L
    CB = C * BL
    nc = bacc.Bacc("TRN2", target_bir_lowering=False, debug=False, num_devices=NC)

    # ---------------- dram I/O ----------------
    ohT = nc.dram_tensor("ohT", [V, TOK], BF, kind="ExternalInput")
    ctxT = nc.dram_tensor("ctxT", [3, 100, TOK], BF, kind="ExternalInput")
    WeT = nc.dram_tensor("WeT", [V, H3], BF, kind="ExternalInput")
    WcT = nc.dram_tensor("WcT", [3, 100, H3], BF, kind="ExternalInput")
    W1T = nc.dram_tensor("W1T", [128, KH, H3], BF, kind="ExternalInput")
    W2T = nc.dram_tensor("W2T", [128, KH, H3], BF, kind="ExternalInput")
    WhT = nc.dram_tensor("WhT", [3, 128, KH, H3], BF, kind="ExternalInput")
    biasP = nc.dram_tensor("biasP", [128, 3, MJ], F32, kind="ExternalInput")
    h0P = nc.dram_tensor("h0P", [128, 3, KH, BL], F32, kind="ExternalInput")
    gamP = nc.dram_tensor("gamP", [128, KH], F32, kind="ExternalInput")
    betP = nc.dram_tensor("betP", [128, KH], F32, kind="ExternalInput")
    WaT = nc.dram_tensor("WaT", [128, KH, H], BF, kind="ExternalInput")
    WcombT = nc.dram_tensor("WcombT", [128, 8, H], BF, kind="ExternalInput")
    bcombP = nc.dram_tensor("bcombP", [128, KH], F32, kind="ExternalInput")
    Wh2eT = nc.dram_tensor("Wh2eT", [128, KH, E], BF, kind="ExternalInput")
    bh2eP = nc.dram_tensor("bh2eP", [128, 1], F32, kind="ExternalInput")
    embT = nc.dram_tensor("embT", [E, V], BF, kind="ExternalInput")
    maskC = nc.dram_tensor("maskC", [128, 1024], BF, kind="ExternalInput")
    onesP = nc.dram_tensor("onesP", [128, 1], BF, kind="ExternalInput")

    logitsT = nc.dram_tensor("logitsT", [V, TOK], F32, kind="ExternalOutput")
    houtP = nc.dram_tensor("houtP", [128, 3, KH, BL], F32, kind="ExternalOutput")

    # internal scratch
    out2 = nc.dram_tensor("out2", [BL, L, H], BF)

    import time
    t0 = time.time()
    with tile.TileContext(nc) as tc:
        _recurrence(nc, tc, locals(), L=L, C=C, U=U, NW=NW, TOK=TOK, CB=CB)
        _attention(nc, tc, locals(), L=L, TOK=TOK)
    t1 = time.time()
    nc.compile()
    print(f"[kernel] trace+tile {t1 - t0:.1f}s bacc-compile {time.time() - t1:.1f}s",
          flush=True)
    return nc


def _recurrence(nc, tc, t_, L, C, U, NW, TOK, CB):
    from contextlib import ExitStack
    ctx = ExitStack()
    with ctx:
        wpool = ctx.enter_context(tc.tile_pool(name="wts", bufs=1))
        inpool = ctx.enter_context(tc.tile_pool(name="inp", bufs=2))
        xipool = ctx.enter_context(tc.tile_pool(name="xi", bufs=1))
        wopool = ctx.enter_context(tc.tile_pool(name="wo", bufs=1))
        gpool = ctx.enter_context(tc.tile_pool(name="gates", bufs=12))
        hpool = ctx.enter_context(tc.tile_pool(name="hstate", bufs=1))
        pspool = ctx.enter_context(tc.tile_pool(name="ps", bufs=8, space="PSUM"))
        ps2pool = pspool

        # ---- load weights ----
        we_sb = wpool.tile([V, H3], BF)
        nc.sync.dma_start(we_sb, t_["WeT"].ap())
        wc_sb = wpool.tile([100, 3, H3], BF)
        nc.sync.dma_start(wc_sb, t_["WcT"].ap().rearrange("c p t -> p c t"))
        wi1_t = wpool.tile([128, KH, H3], BF, tag="w1")
        wi2_t = wpool.tile([128, KH, H3], BF, tag="w2")
        wi_sb = {1: wi1_t, 2: wi2_t}
        nc.sync.dma_start(wi_sb[1], t_["W1T"].ap())
        nc.sync.dma_start(wi_sb[2], t_["W2T"].ap())
        wh_sb = []
        for l in range(3):
            wh_t = wpool.tile([128, KH, H3], BF, tag=f"wh{l}")
            wh_sb.append(wh_t)
            nc.sync.dma_start(wh_t, t_["WhT"].ap()[l])
        bias_sb = wpool.tile([128, 3, MJ], F32)
        nc.sync.dma_start(bias_sb, t_["biasP"].ap())
        h_all = hpool.tile([128, 3, KH, BL], F32)
        nc.sync.dma_start(h_all, t_["h0P"].ap())

        # persistent per-layer state
        xi_sb, wout, h_b = [], [], []
        for l in range(3):
            xi_t = xipool.tile([128, MJ, CB], BF, tag=f"xis{l}")
            xi_sb.append(xi_t)
            wo_t = wopool.tile([128, KH, CB], BF, tag=f"wob{l}")
            wout.append(wo_t)
            hb_t = hpool.tile([128, KH, BL], BF, tag=f"hb{l}")
            h_b.append(hb_t)
            nc.vector.tensor_copy(hb_t, h_all[:, l])

        NCH = min(512, CB)
        # wave-pipelined over layers: at tick wt, layer l works on wave wt-l
        for wt in range(NW + 2):
            active = [l for l in range(3) if 0 <= wt - l < NW]
            # ---- PRE: layer-0 xi from onehot+context (wave wt) ----
            if 0 in active:
                w = wt
                oh_t = inpool.tile([V, CB], BF, tag="ohsl")
                nc.sync.dma_start(oh_t, t_["ohT"].ap()[:, ts(w, CB)])
                ctx_t = inpool.tile([100, 3, CB], BF, tag="ctxsl")
                nc.sync.dma_start(
                    ctx_t,
                    t_["ctxT"].ap().rearrange("c p t -> p c t")[:, :, ts(w, CB)])
                for j in range(MJ):
                    for nh in range(CB // NCH):
                        pt = ps2pool.tile([128, NCH], F32, tag="ps")
                        nc.tensor.matmul(pt, we_sb[:, ts(j, 128)],
                                         oh_t[:, ds(nh * NCH, NCH)],
                                         start=True, stop=False)
                        for cc in range(3):
                            nc.tensor.matmul(pt, wc_sb[:, cc, ts(j, 128)],
                                             ctx_t[:, cc, ds(nh * NCH, NCH)],
                                             start=False, stop=(cc == 2))
                        nc.scalar.activation(xi_sb[0][:, j, ts(nh, NCH)], pt,
                                             AF.Identity,
                                             bias=bias_sb[:, 0, j:j + 1])

            # ---- LOOP: interleaved steps of all active layers ----
            with tc.For_i(0, C, U,
                          hint_engines=(mybir.EngineType.PE,)) as i0:
                for u in range(U):
                    tsl = ds(i0 * BL + u * BL, BL)
                    for l in active:
                        pt = pspool.tile([128, 8, BL], F32, tag="ps")
                        ptn = pspool.tile([128, KH, BL], F32, tag="ps")
                        for j in range(MJ):
                            dst = pt[:, j] if j < 8 else ptn[:, j - 8]
                            for k in range(KH):
                                nc.tensor.matmul(
                                    dst, wh_sb[l][:, k, ts(j, 128)],
                                    h_b[l][:, k], start=(k == 0),
                                    stop=(k == KH - 1))
                        xi_t = xi_sb[l][:, :, tsl]
                        trz = gpool.tile([128, 8, BL], F32, tag="trz")
                        nc.vector.tensor_tensor(trz, pt, xi_t[:, 0:8], op=OP.add)
                        rz = gpool.tile([128, 8, BL], F32, tag="rz")
                        nc.scalar.activation(rz, trz, AF.Sigmoid)
                        tn = gpool.tile([128, KH, BL], F32, tag="tn")
                        nc.vector.tensor_tensor(tn, rz[:, 0:KH], ptn, op=OP.mult)
                        tn2 = gpool.tile([128, KH, BL], F32, tag="tn2")
                        nc.vector.tensor_tensor(tn2, tn, xi_t[:, 8:MJ], op=OP.add)
                        nn = gpool.tile([128, KH, BL], F32, tag="nn")
                        nc.scalar.activation(nn, tn2, AF.Tanh)
                        dd = gpool.tile([128, KH, BL], F32, tag="dd")
                        nc.vector.tensor_sub(dd, h_b[l], nn)
                        ee = gpool.tile([128, KH, BL], F32, tag="ee")
                        nc.vector.tensor_tensor(ee, rz[:, KH:8], dd, op=OP.mult)
                        nc.vector.tensor_add(h_b[l], nn, ee)
                        nc.vector.tensor_copy(wout[l][:, :, tsl], h_b[l])

            # ---- POST: ship wave outputs ----
            for l in active:
                w = wt - l
                if l < 2:
                    for j in range(MJ):
                        for nh in range(CB // NCH):
                            pt = ps2pool.tile([128, NCH], F32, tag="ps")
                            for k in range(KH):
                                nc.tensor.matmul(
                                    pt, wi_sb[l + 1][:, k, ts(j, 128)],
                                    wout[l][:, k, ts(nh, NCH)], start=(k == 0),
                                    stop=(k == KH - 1))
                            nc.scalar.activation(
                                xi_sb[l + 1][:, j, ts(nh, NCH)], pt,
                                AF.Identity, bias=bias_sb[:, l + 1, j:j + 1])
                else:
                    for b in range(BL):
                        for k in range(KH):
                            src = wout[2][:, k].rearrange(
                                "p (t b) -> p t b", b=BL)[:, :, b]
                            dst = t_["out2"].ap()[b, ts(w, C)].rearrange(
                                "t (k p) -> p k t", p=128)[:, k]
                            nc.sync.dma_start(dst, src)

        for l in range(3):
            nc.vector.tensor_copy(h_all[:, l], h_b[l])
        nc.sync.dma_start(t_["houtP"].ap(), h_all)


def _attention(nc, tc, t_, L, TOK):
    from contextlib import ExitStack
    QT = 512 if L >= 512 else L
    NQ = L // QT    # q tiles per batch elem
    NKT = L // 128  # key tiles per batch elem
    SC = 1.0 / np.sqrt(float(H))
    ctx = ExitStack()
    with ctx:
        wpool = ctx.enter_context(tc.tile_pool(name="awts", bufs=1))
        lpool = ctx.enter_context(tc.tile_pool(name="aln", bufs=2))
        kpool = ctx.enter_context(tc.tile_pool(name="akv", bufs=1))
        spool = ctx.enter_context(tc.tile_pool(name="asc", bufs=3))
        opool = ctx.enter_context(tc.tile_pool(name="aout", bufs=2))
        pspool = ctx.enter_context(tc.tile_pool(name="aps", bufs=3, space="PSUM"))
        pvpool = ctx.enter_context(tc.tile_pool(name="apv", bufs=1, space="PSUM"))

        gam = wpool.tile([128, KH], F32)
        nc.sync.dma_start(gam, t_["gamP"].ap())
        bet = wpool.tile([128, KH], F32)
        nc.sync.dma_start(bet, t_["betP"].ap())
        wa = wpool.tile([128, KH, H], BF)
        nc.sync.dma_start(wa, t_["WaT"].ap())
        wcb = wpool.tile([128, 8, H], BF)
        nc.sync.dma_start(wcb, t_["WcombT"].ap())
        bcb = wpool.tile([128, KH], F32)
        nc.sync.dma_start(bcb, t_["bcombP"].ap())
        wh2e = wpool.tile([128, KH, E], BF)
        nc.sync.dma_start(wh2e, t_["Wh2eT"].ap())
        bh2e = wpool.tile([128, 1], F32)
        nc.sync.dma_start(bh2e, t_["bh2eP"].ap())
        emb = wpool.tile([E, V], BF)
        nc.sync.dma_start(emb, t_["embT"].ap())
        msk = wpool.tile([128, 1024], BF)
        nc.sync.dma_start(msk, t_["maskC"].ap())
        ones = wpool.tile([128, 1], BF)
        nc.sync.dma_start(ones, t_["onesP"].ap())
        ident = wpool.tile([128, 128], BF)
        make_identity(nc, ident)
        epsT = wpool.tile([128, 1], F32)
        nc.vector.memset(epsT, 1e-5)

        for b in range(BL):
            # ---- LayerNorm (plain) + transpose; build zn rows + znT ----
            zn = kpool.tile([128, NKT, H], BF, tag="zn")       # rows (tok,h)
            znT = kpool.tile([128, KH, L], BF, tag="znT")      # (h, tok)
            for kt in range(NKT):
                xr = lpool.tile([128, H], BF, tag="xrow")
                nc.sync.dma_start(
                    xr, t_["out2"].ap()[b, ts(kt, 128)])
                st = lpool.tile([128, 6], F32, tag="stats")
                mv = lpool.tile([128, 2], F32, tag="mv")
                nc.vector.bn_stats(st, xr)
                nc.vector.bn_aggr(mv, st)
                rstd = lpool.tile([128, 1], F32, tag="rstd")
                nc.scalar.activation(rstd, mv[:, 1:2], AF.Sqrt, bias=epsT)
                nc.vector.reciprocal(rstd, rstd)
                xc = lpool.tile([128, H], F32, tag="xc")
                nc.vector.tensor_scalar(xc, xr, mv[:, 0:1], rstd,
                                        op0=OP.subtract, op1=OP.mult)
                nc.vector.tensor_copy(zn[:, kt], xc)
                # transpose 4 128x128 blocks into znT
                for k in range(KH):
                    ptt = pspool.tile([128, 128], BF, tag="ps512")
                    nc.tensor.transpose(ptt, zn[:, kt, ts(k, 128)], ident)
                    nc.vector.tensor_copy(znT[:, k, ts(kt, 128)], ptt)
            # affined query/key view
            znTa = kpool.tile([128, KH, L], BF, tag="znTa")
            for k in range(KH):
                nc.vector.tensor_scalar(znTa[:, k], znT[:, k],
                                        gam[:, k:k + 1], bet[:, k:k + 1],
                                        op0=OP.mult, op1=OP.add)
            # keys.T = Wa @ znTa : (h', tok)
            keT = kpool.tile([128, KH, L], BF, tag="keT")
            for hc in range(KH):
                for qt4 in range(L // QT):
                    pk = pspool.tile([128, QT], F32, tag="ps512")
                    for k in range(KH):
                        nc.tensor.matmul(pk, wa[:, k, ts(hc, 128)],
                                         znTa[:, k, ts(qt4, QT)],
                                         start=(k == 0), stop=(k == KH - 1))
                    nc.vector.tensor_copy(keT[:, hc, ts(qt4, QT)], pk)

            for qi in range(NQ):
                pvs = []
                for hc in range(KH):
                    pv_t = pvpool.tile([128, QT], F32, tag=f"pv{hc}")
                    pvs.append(pv_t)
                lsum = pvpool.tile([1, QT], F32, tag="ls")
                nkt = (QT // 128) * (qi + 1)
                for kt in range(nkt):
                    pss = pspool.tile([128, QT], F32, tag="ps512")
                    for k in range(KH):
                        nc.tensor.matmul(pss, keT[:, k, ts(kt, 128)],
                                         znTa[:, k, ts(qi, QT)],
                                         start=(k == 0), stop=(k == KH - 1))
                    es = spool.tile([128, QT], BF, tag="es")
                    nc.scalar.activation(es, pss, AF.Exp, scale=SC)
                    d = QT * qi - 128 * kt
                    if d < 128:  # diagonal tile -> mask (keep q >= key)
                        es2 = spool.tile([128, QT], BF, tag="es2")
                        nc.vector.tensor_tensor(
                            es2, es, msk[:, 512 + d:512 + d + QT], op=OP.mult)
                        es = es2
                    first = kt == 0
                    last = kt == nkt - 1
                    for hc in range(KH):
                        nc.tensor.matmul(pvs[hc], zn[:, kt, ts(hc, 128)], es,
                                         start=first, stop=last)
                    nc.tensor.matmul(lsum, ones, es, start=first, stop=last)
                # normalize + affine the context
                rl = spool.tile([1, QT], F32, tag="rl")
                nc.vector.reciprocal(rl, lsum)
                rlb = spool.tile([128, QT], F32, tag="rlb")
                nc.gpsimd.partition_broadcast(rlb, rl)
                cta = opool.tile([128, KH, QT], BF, tag="cta")
                for hc in range(KH):
                    cn = spool.tile([128, QT], F32, tag="cn")
                    nc.vector.tensor_tensor(cn, pvs[hc], rlb, op=OP.mult)
                    nc.vector.tensor_scalar(cta[:, hc], cn,
                                            gam[:, hc:hc + 1], bet[:, hc:hc + 1],
                                            op0=OP.mult, op1=OP.add)
                # combined = tanh(Wcomb @ [znTa; cta] + b)
                cmb = opool.tile([128, KH, QT], BF, tag="cmb")
                for hc in range(KH):
                    pc = pspool.tile([128, QT], F32, tag="ps512")
                    for k in range(8):
                        rhs = znTa[:, k, ts(qi, QT)] if k < KH else cta[:, k - KH]
                        nc.tensor.matmul(pc, wcb[:, k, ts(hc, 128)], rhs,
                                         start=(k == 0), stop=(k == 7))
                    nc.scalar.activation(cmb[:, hc], pc, AF.Tanh,
                                         bias=bcb[:, hc:hc + 1])
                # emb_space.T = Wh2e @ cmb ; logits.T = embT.T @ emb_space.T
                pe = pspool.tile([128, QT], F32, tag="ps512")
                for k in range(KH):
                    nc.tensor.matmul(pe, wh2e[:, k], cmb[:, k],
                                     start=(k == 0), stop=(k == KH - 1))
                esb = opool.tile([128, QT], BF, tag="esb")
                nc.scalar.activation(esb, pe, AF.Identity, bias=bh2e[:, 0:1])
                pl = pspool.tile([V, QT], F32, tag="ps512")
                nc.tensor.matmul(pl, emb, esb, start=True, stop=True)
                lo = opool.tile([V, QT], F32, tag="lo")
                nc.vector.tensor_copy(lo, pl)
                nc.sync.dma_start(
                    t_["logitsT"].ap().rearrange("v (t b) -> v t b", b=BL)[
                        :, ts(qi, QT), b], lo)


def _host_prep(x, context, h, embedding, W_ctx, b_ctx,
               Wi0, Wh0, bi0, bh0, Wi1, Wh1, bi1, bh1, Wi2, Wh2, bi2, bh2,
               ln_g, ln_b, Wa, W_comb, b_comb, W_h2e, b_h2e, L):
    TOK = BL * L
    f = np.float32
    x = np.asarray(x)
    # folded layer-0 weights
    WeT = (embedding.astype(f) @ Wi0[:, :E].astype(f).T)          # (72, 1536)
    Wc = (Wi0[:, E:].astype(f) @ W_ctx.astype(f))                 # (1536, 300)
    bias0 = bi0 + bh0 + Wi0[:, E:].astype(f) @ b_ctx.astype(f)
    bias1 = bi1 + bh1
    bias2 = bi2 + bh2

    def to_bf(a):
        return np.ascontiguousarray(a).astype(NPBF)

    shared = dict(
        WeT=to_bf(WeT),
        WcT=to_bf(Wc.T.reshape(3, 100, H3)),
        W1T=to_bf(Wi1.T.reshape(KH, 128, H3).transpose(1, 0, 2)),
        W2T=to_bf(Wi2.T.reshape(KH, 128, H3).transpose(1, 0, 2)),
        WhT=to_bf(np.stack([W.T.reshape(KH, 128, H3).transpose(1, 0, 2)
                            for W in (Wh0, Wh1, Wh2)])),
        biasP=np.stack([b.reshape(MJ, 128).T for b in (bias0, bias1, bias2)],
                       axis=1).astype(f),
        gamP=ln_g.reshape(KH, 128).T.astype(f),
        betP=ln_b.reshape(KH, 128).T.astype(f),
        WaT=to_bf(Wa.T.reshape(KH, 128, H).transpose(1, 0, 2)),
        WcombT=to_bf(W_comb.T.reshape(8, 128, H).transpose(1, 0, 2)),
        bcombP=b_comb.reshape(KH, 128).T.astype(f),
        Wh2eT=to_bf(W_h2e.T.reshape(KH, 128, E).transpose(1, 0, 2)),
        bh2eP=b_h2e.reshape(1, 128).T.astype(f),
        embT=to_bf(embedding.T),
        maskC=((np.arange(1024)[None, :] - 512) >= np.arange(128)[:, None]
               ).astype(NPBF),
        onesP=np.ones((128, 1), NPBF),
    )
    in_maps = []
    for c in range(NC):
        bs = slice(c * BL, (c + 1) * BL)
        xs = np.asarray(x[bs])                                   # (BL, L)
        oh = (xs[:, :, None] == np.arange(V)).astype(NPBF)       # (BL,L,V)
        ohT = np.ascontiguousarray(oh.transpose(2, 1, 0)).reshape(V, TOK)
        cs = np.asarray(context[bs], f).reshape(BL, L, 300)
        ctxT = np.ascontiguousarray(cs.transpose(2, 1, 0)).reshape(
            3, 100, TOK).astype(NPBF)
        h0 = np.asarray(h[:, bs], f)                             # (3, BL, H)
        h0P = np.ascontiguousarray(
            h0.reshape(3, BL, KH, 128).transpose(3, 0, 2, 1))
        m = dict(shared)
        m.update(ohT=ohT, ctxT=ctxT, h0P=h0P)
        in_maps.append(m)
    return in_maps


def kernel(x, context, h, embedding, W_ctx, b_ctx,
           Wi0, Wh0, bi0, bh0, Wi1, Wh1, bi1, bh1, Wi2, Wh2, bi2, bh2,
           ln_g, ln_b, Wa, W_comb, b_comb, W_h2e, b_h2e):
    L = np.asarray(x).shape[1]
    assert L == _CFG["L"], f"configure _set_mode for L={L}"
    key = "prog"
    if key not in _BUILT:
        _BUILT[key] = _build_program()
    nc = _BUILT[key]
    args = dict(x=x, context=context, h=h, embedding=embedding, W_ctx=W_ctx,
                b_ctx=b_ctx, Wi0=Wi0, Wh0=Wh0, bi0=bi0, bh0=bh0, Wi1=Wi1,
                Wh1=Wh1, bi1=bi1, bh1=bh1, Wi2=Wi2, Wh2=Wh2, bi2=bi2,
                bh2=bh2, ln_g=ln_g, ln_b=ln_b, Wa=Wa, W_comb=W_comb,
                b_comb=b_comb, W_h2e=W_h2e, b_h2e=b_h2e)
    args = {k: np.asarray(v) for k, v in args.items()}
    in_maps = _host_prep(L=L, **args)
    res = run_bass_kernel_spmd(nc, in_maps, core_ids=list(range(NC)))
    global _LAST_RES
    _LAST_RES = res.results
    logits = np.empty((B, L, V), np.float32)
    h_out = np.empty((3, B, H), np.float32)
    for c in range(NC):
        lt = res.results[c]["logitsT"].reshape(V, L, BL)
        logits[c * BL:(c + 1) * BL] = lt.transpose(2, 1, 0)
        hp = res.results[c]["houtP"]                              # (128,3,KH,BL)
        h_out[:, c * BL:(c + 1) * BL] = hp.transpose(1, 3, 2, 0).reshape(
            3, BL, H)
    return logits, h_out


# revision 42
# speedup vs baseline: 1.0462x; 1.0462x over previous
"""Char-GRU (3-layer GRU + causal attention LM) Trainium2 kernel.

Sharding: data-parallel over batch. B=16 across 8 cores -> 2 batch elems/core.

Device layouts (per core, BL=2 local batch, token index tok = t*BL + b):
  - GRU state h:         (128, KH, BL)  partition p = h%128, k = h//128
  - xi (input gates):    (128, MJ, C*BL) per wave; m-chunk j = (3H)//128
  - wave output out_l.T: (128, KH, C*BL)
  - attention:           scores.T (kt-part, q-free); everything H-on-partition
"""

import numpy as np
import ml_dtypes

import concourse.bass as bass
import concourse.tile as tile
from concourse import bacc, mybir
from concourse.bass import ds, ts
from concourse.bass_utils import run_bass_kernel_spmd
from concourse.masks import make_identity

BF = mybir.dt.bfloat16
F32 = mybir.dt.float32
AF = mybir.ActivationFunctionType
OP = mybir.AluOpType
NPBF = ml_dtypes.bfloat16

NC = 8
B, V, E, H = 16, 72, 128, 512
BL = B // NC
H3 = 3 * H
KH = H // 128   # 4
MJ = H3 // 128  # 12

# full-size config; test.py may call _set_mode for a small smoke config
_CFG = dict(L=2048, C=512, U=8)


def _set_mode(L, C, U):
    _CFG.update(L=L, C=C, U=U)
    _BUILT.clear()


_BUILT = {}
_LAST_RES = None


def _build_program():
    L, C, U = _CFG["L"], _CFG["C"], _CFG["U"]
    NW = L // C
    TOK = BL * L
    CB = C * BL
    nc = bacc.Bacc("TRN2", target_bir_lowering=False, debug=False, num_devices=NC)

    # ---------------- dram I/O ----------------
    ohT = nc.dram_tensor("ohT", [V, TOK], BF, kind="ExternalInput")
    ctxT = nc.dram_tensor("ctxT", [3, 100, TOK], BF, kind="ExternalInput")
    WeT = nc.dram_tensor("WeT", [V, H3], BF, kind="ExternalInput")
    WcT = nc.dram_tensor("WcT", [3, 100, H3], BF, kind="ExternalInput")
    W1T = nc.dram_tensor("W1T", [128, KH, H3], BF, kind="ExternalInput")
    W2T = nc.dram_tensor("W2T", [128, KH, H3], BF, kind="ExternalInput")
    WhT = nc.dram_tensor("WhT", [3, 128, KH, H3], BF, kind="ExternalInput")
    biasP = nc.dram_tensor("biasP", [128, 3, MJ], F32, kind="ExternalInput")
    h0P = nc.dram_tensor("h0P", [128, 3, KH, BL], F32, kind="ExternalInput")
    gamP = nc.dram_tensor("gamP", [128, KH], F32, kind="ExternalInput")
    betP = nc.dram_tensor("betP", [128, KH], F32, kind="ExternalInput")
    WaT = nc.dram_tensor("WaT", [128, KH, H], BF, kind="ExternalInput")
    WcombT = nc.dram_tensor("WcombT", [128, 8, H], BF, kind="ExternalInput")
    bcombP = nc.dram_tensor("bcombP", [128, KH], F32, kind="ExternalInput")
    Wh2eT = nc.dram_tensor("Wh2eT", [128, KH, E], BF, kind="ExternalInput")
    bh2eP = nc.dram_tensor("bh2eP", [128, 1], F32, kind="ExternalInput")
    embT = nc.dram_tensor("embT", [E, V], BF, kind="ExternalInput")
    maskC = nc.dram_tensor("maskC", [128, 1024], BF, kind="ExternalInput")
    onesP = nc.dram_tensor("onesP", [128, 1], BF, kind="ExternalInput")

    logitsT = nc.dram_tensor("logitsT", [V, TOK], F32, kind="ExternalOutput")
    houtP = nc.dram_tensor("houtP", [128, 3, KH, BL], F32, kind="ExternalOutput")

    # internal scratch
    out2 = nc.dram_tensor("out2", [BL, L, H], BF)

    import time
    t0 = time.time()
    with tile.TileContext(nc) as tc:
        _recurrence(nc, tc, locals(), L=L, C=C, U=U, NW=NW, TOK=TOK, CB=CB)
        _attention(nc, tc, locals(), L=L, TOK=TOK)
    t1 = time.time()
    nc.compile()
    print(f"[kernel] trace+tile {t1 - t0:.1f}s bacc-compile {time.time() - t1:.1f}s",
          flush=True)
    return nc


def _recurrence(nc, tc, t_, L, C, U, NW, TOK, CB):
    from contextlib import ExitStack
    ctx = ExitStack()
    with ctx:
        wpool = ctx.enter_context(tc.tile_pool(name="wts", bufs=1))
        inpool = ctx.enter_context(tc.tile_pool(name="inp", bufs=2))
        xipool = ctx.enter_context(tc.tile_pool(name="xi", bufs=1))
        wopool = ctx.enter_context(tc.tile_pool(name="wo", bufs=1))
        gpool = ctx.enter_context(tc.tile_pool(name="gates", bufs=6))
        hpool = ctx.enter_context(tc.tile_pool(name="hstate", bufs=1))
        pspool = ctx.enter_context(tc.tile_pool(name="ps", bufs=3, space="PSUM"))
        ps2pool = ctx.enter_context(tc.tile_pool(name="ps2", bufs=2, space="PSUM"))

        # ---- load weights ----
        we_sb = wpool.tile([V, H3], BF)
        nc.sync.dma_start(we_sb, t_["WeT"].ap())
        wc_sb = wpool.tile([100, 3, H3], BF)
        nc.sync.dma_start(wc_sb, t_["WcT"].ap().rearrange("c p t -> p c t"))
        wi1_t = wpool.tile([128, KH, H3], BF, tag="w1")
        wi2_t = wpool.tile([128, KH, H3], BF, tag="w2")
        wi_sb = {1: wi1_t, 2: wi2_t}
        nc.sync.dma_start(wi_sb[1], t_["W1T"].ap())
        nc.sync.dma_start(wi_sb[2], t_["W2T"].ap())
        wh_sb = []
        for l in range(3):
            wh_t = wpool.tile([128, KH, H3], BF, tag=f"wh{l}")
            wh_sb.append(wh_t)
            nc.sync.dma_start(wh_t, t_["WhT"].ap()[l])
        bias_sb = wpool.tile([128, 3, MJ], F32)
        nc.sync.dma_start(bias_sb, t_["biasP"].ap())
        h_all = hpool.tile([128, 3, KH, BL], F32)
        nc.sync.dma_start(h_all, t_["h0P"].ap())

        # persistent per-layer state
        xi_sb, wout, h_b = [], [], []
        for l in range(3):
            xi_t = xipool.tile([128, MJ, CB], BF, tag=f"xis{l}")
            xi_sb.append(xi_t)
            wo_t = wopool.tile([128, KH, CB], BF, tag=f"wob{l}")
            wout.append(wo_t)
            hb_t = hpool.tile([128, KH, BL], BF, tag=f"hb{l}")
            h_b.append(hb_t)
            nc.vector.tensor_copy(hb_t, h_all[:, l])

        NCH = min(512, CB)
        # wave-pipelined over layers: at tick wt, layer l works on wave wt-l
        for wt in range(NW + 2):
            active = [l for l in range(3) if 0 <= wt - l < NW]
            # ---- PRE: layer-0 xi from onehot+context (wave wt) ----
            if 0 in active:
                w = wt
                oh_t = inpool.tile([V, CB], BF, tag="ohsl")
                nc.sync.dma_start(oh_t, t_["ohT"].ap()[:, ts(w, CB)])
                ctx_t = inpool.tile([100, 3, CB], BF, tag="ctxsl")
                nc.sync.dma_start(
                    ctx_t,
                    t_["ctxT"].ap().rearrange("c p t -> p c t")[:, :, ts(w, CB)])
                for j in range(MJ):
                    for nh in range(CB // NCH):
                        pt = ps2pool.tile([128, NCH], F32, tag="xips")
                        nc.tensor.matmul(pt, we_sb[:, ts(j, 128)],
                                         oh_t[:, ds(nh * NCH, NCH)],
                                         start=True, stop=False)
                        for cc in range(3):
                            nc.tensor.matmul(pt, wc_sb[:, cc, ts(j, 128)],
                                             ctx_t[:, cc, ds(nh * NCH, NCH)],
                                             start=False, stop=(cc == 2))
                        nc.scalar.activation(xi_sb[0][:, j, ts(nh, NCH)], pt,
                                             AF.Identity,
                                             bias=bias_sb[:, 0, j:j + 1])

            # ---- LOOP: interleaved steps of all active layers ----
            with tc.For_i(0, C, U,
                          hint_engines=(mybir.EngineType.PE,)) as i0:
                for u in range(U):
                    tsl = ds(i0 * BL + u * BL, BL)
                    for l in active:
                        pt = pspool.tile([128, 8, BL], F32, tag="rec")
                        ptn = pspool.tile([128, KH, BL], F32, tag="recn")
                        for j in range(MJ):
                            dst = pt[:, j] if j < 8 else ptn[:, j - 8]
                            for k in range(KH):
                                nc.tensor.matmul(
                                    dst, wh_sb[l][:, k, ts(j, 128)],
                                    h_b[l][:, k], start=(k == 0),
                                    stop=(k == KH - 1))
                        xi_t = xi_sb[l][:, :, tsl]
                        trz = gpool.tile([128, 8, BL], F32, tag="trz")
                        nc.vector.tensor_tensor(trz, pt, xi_t[:, 0:8], op=OP.add)
                        rz = gpool.tile([128, 8, BL], F32, tag="rz")
                        nc.scalar.activation(rz, trz, AF.Sigmoid)
                        tn = gpool.tile([128, KH, BL], F32, tag="tn")
                        nc.vector.tensor_tensor(tn, rz[:, 0:KH], ptn, op=OP.mult)
                        tn2 = gpool.tile([128, KH, BL], F32, tag="tn2")
                        nc.vector.tensor_tensor(tn2, tn, xi_t[:, 8:MJ], op=OP.add)
                        nn = gpool.tile([128, KH, BL], F32, tag="nn")
                        nc.scalar.activation(nn, tn2, AF.Tanh)
                        dd = gpool.tile([128, KH, BL], F32, tag="dd")
                        nc.vector.tensor_sub(dd, h_b[l], nn)
                        ee = gpool.tile([128, KH, BL], F32, tag="ee")
                        nc.vector.tensor_tensor(ee, rz[:, KH:8], dd, op=OP.mult)
                        nc.vector.tensor_add(h_b[l], nn, ee)
                        nc.vector.tensor_copy(wout[l][:, :, tsl], h_b[l])

            # ---- POST: ship wave outputs ----
            for l in active:
                w = wt - l
                if l < 2:
                    for j in range(MJ):
                        for nh in range(CB // NCH):
                            pt = ps2pool.tile([128, NCH], F32, tag="xips")
                            for k in range(KH):
                                nc.tensor.matmul(
                                    pt, wi_sb[l + 1][:, k, ts(j, 128)],
                                    wout[l][:, k, ts(nh, NCH)], start=(k == 0),
                                    stop=(k == KH - 1))
                            nc.scalar.activation(
                                xi_sb[l + 1][:, j, ts(nh, NCH)], pt,
                                AF.Identity, bias=bias_sb[:, l + 1, j:j + 1])
                else:
                    for b in range(BL):
                        for k in range(KH):
                            src = wout[2][:, k].rearrange(
                                "p (t b) -> p t b", b=BL)[:, :, b]
                            dst = t_["out2"].ap()[b, ts(w, C)].rearrange(
                                "t (k p) -> p k t", p=128)[:, k]
                            nc.sync.dma_start(dst, src)

        for l in range(3):
            nc.vector.tensor_copy(h_all[:, l], h_b[l])
        nc.sync.dma_start(t_["houtP"].ap(), h_all)


def _attention(nc, tc, t_, L, TOK):
    from contextlib import ExitStack
    QT = 512 if L >= 512 else L
    NQ = L // QT    # q tiles per batch elem
    NKT = L // 128  # key tiles per batch elem
    SC = 1.0 / np.sqrt(float(H))
    ctx = ExitStack()
    with ctx:
        wpool = ctx.enter_context(tc.tile_pool(name="awts", bufs=1))
        lpool = ctx.enter_context(tc.tile_pool(name="aln", bufs=2))
        kpool = ctx.enter_context(tc.tile_pool(name="akv", bufs=1))
        spool = ctx.enter_context(tc.tile_pool(name="asc", bufs=3))
        opool = ctx.enter_context(tc.tile_pool(name="aout", bufs=2))
        pspool = ctx.enter_context(tc.tile_pool(name="aps", bufs=3, space="PSUM"))
        pvpool = ctx.enter_context(tc.tile_pool(name="apv", bufs=1, space="PSUM"))

        gam = wpool.tile([128, KH], F32)
        nc.sync.dma_start(gam, t_["gamP"].ap())
        bet = wpool.tile([128, KH], F32)
        nc.sync.dma_start(bet, t_["betP"].ap())
        wa = wpool.tile([128, KH, H], BF)
        nc.sync.dma_start(wa, t_["WaT"].ap())
        wcb = wpool.tile([128, 8, H], BF)
        nc.sync.dma_start(wcb, t_["WcombT"].ap())
        bcb = wpool.tile([128, KH], F32)
        nc.sync.dma_start(bcb, t_["bcombP"].ap())
        wh2e = wpool.tile([128, KH, E], BF)
        nc.sync.dma_start(wh2e, t_["Wh2eT"].ap())
        bh2e = wpool.tile([128, 1], F32)
        nc.sync.dma_start(bh2e, t_["bh2eP"].ap())
        emb = wpool.tile([E, V], BF)
        nc.sync.dma_start(emb, t_["embT"].ap())
        msk = wpool.tile([128, 1024], BF)
        nc.sync.dma_start(msk, t_["maskC"].ap())
        ones = wpool.tile([128, 1], BF)
        nc.sync.dma_start(ones, t_["onesP"].ap())
        ident = wpool.tile([128, 128], BF)
        make_identity(nc, ident)
        epsT = wpool.tile([128, 1], F32)
        nc.vector.memset(epsT, 1e-5)

        for b in range(BL):
            # ---- LayerNorm (plain) + transpose; build zn rows + znT ----
            zn = kpool.tile([128, NKT, H], BF, tag="zn")       # rows (tok,h)
            znT = kpool.tile([128, KH, L], BF, tag="znT")      # (h, tok)
            for kt in range(NKT):
                xr = lpool.tile([128, H], BF, tag="xrow")
                nc.sync.dma_start(
                    xr, t_["out2"].ap()[b, ts(kt, 128)])
                st = lpool.tile([128, 6], F32, tag="stats")
                mv = lpool.tile([128, 2], F32, tag="mv")
                nc.vector.bn_stats(st, xr)
                nc.vector.bn_aggr(mv, st)
                rstd = lpool.tile([128, 1], F32, tag="rstd")
                nc.scalar.activation(rstd, mv[:, 1:2], AF.Sqrt, bias=epsT)
                nc.vector.reciprocal(rstd, rstd)
                xc = lpool.tile([128, H], F32, tag="xc")
                nc.vector.tensor_scalar(xc, xr, mv[:, 0:1], rstd,
                                        op0=OP.subtract, op1=OP.mult)
                nc.vector.tensor_copy(zn[:, kt], xc)
                # transpose 4 128x128 blocks into znT
                for k in range(KH):
                    ptt = pspool.tile([128, 128], BF, tag="ps512")
                    nc.tensor.transpose(ptt, zn[:, kt, ts(k, 128)], ident)
                    nc.vector.tensor_copy(znT[:, k, ts(kt, 128)], ptt)
            # affined query/key view
            znTa = kpool.tile([128, KH, L], BF, tag="znTa")
            for k in range(KH):
                nc.vector.tensor_scalar(znTa[:, k], znT[:, k],
                                        gam[:, k:k + 1], bet[:, k:k + 1],
                                        op0=OP.mult, op1=OP.add)
            # keys.T = Wa @ znTa : (h', tok)
            keT = kpool.tile([128, KH, L], BF, tag="keT")
            for hc in range(KH):
                for qt4 in range(L // QT):
                    pk = pspool.tile([128, QT], F32, tag="ps512")
                    for k in range(KH):
                        nc.tensor.matmul(pk, wa[:, k, ts(hc, 128)],
                                         znTa[:, k, ts(qt4, QT)],
                                         start=(k == 0), stop=(k == KH - 1))
                    nc.vector.tensor_copy(keT[:, hc, ts(qt4, QT)], pk)

            for qi in range(NQ):
                pvs = []
                for hc in range(KH):
                    pv_t = pvpool.tile([128, QT], F32, tag=f"pv{hc}")
                    pvs.append(pv_t)
                lsum = pvpool.tile([1, QT], F32, tag="ls")
                nkt = (QT // 128) * (qi + 1)
                for kt in range(nkt):
                    pss = pspool.tile([128, QT], F32, tag="ps512")
                    for k in range(KH):
                        nc.tensor.matmul(pss, keT[:, k, ts(kt, 128)],
                                         znTa[:, k, ts(qi, QT)],
                                         start=(k == 0), stop=(k == KH - 1))
                    es = spool.tile([128, QT], BF, tag="es")
                    nc.scalar.activation(es, pss, AF.Exp, scale=SC)
                    d = QT * qi - 128 * kt
                    if d < 128:  # diagonal tile -> mask (keep q >= key)
                        es2 = spool.tile([128, QT], BF, tag="es2")
                        nc.vector.tensor_tensor(
                            es2, es, msk[:, 512 + d:512 + d + QT], op=OP.mult)
                        es = es2
                    first = kt == 0
                    last = kt == nkt - 1
                    for hc in range(KH):
                        nc.tensor.matmul(pvs[hc], zn[:, kt, ts(hc, 128)], es,
                                         start=first, stop=last)
                    nc.tensor.matmul(lsum, ones, es, start=first, stop=last)
                # normalize + affine the context
                rl = spool.tile([1, QT], F32, tag="rl")
                nc.vector.reciprocal(rl, lsum)
                rlb = spool.tile([128, QT], F32, tag="rlb")
                nc.gpsimd.partition_broadcast(rlb, rl)
                cta = opool.tile([128, KH, QT], BF, tag="cta")
                for hc in range(KH):
                    cn = spool.tile([128, QT], F32, tag="cn")
                    nc.vector.tensor_tensor(cn, pvs[hc], rlb, op=OP.mult)
                    nc.vector.tensor_scalar(cta[:, hc], cn,
                                            gam[:, hc:hc + 1], bet[:, hc:hc + 1],
                                            op0=OP.mult, op1=OP.add)
                # combined = tanh(Wcomb @ [znTa; cta] + b)
                cmb = opool.tile([128, KH, QT], BF, tag="cmb")
                for hc in range(KH):
                    pc = pspool.tile([128, QT], F32, tag="ps512")
                    for k in range(8):
                        rhs = znTa[:, k, ts(qi, QT)] if k < KH else cta[:, k - KH]
                        nc.tensor.matmul(pc, wcb[:, k, ts(hc, 128)], rhs,
                                         start=(k == 0), stop=(k == 7))
                    nc.scalar.activation(cmb[:, hc], pc, AF.Tanh,
                                         bias=bcb[:, hc:hc + 1])
                # emb_space.T = Wh2e @ cmb ; logits.T = embT.T @ emb_space.T
                pe = pspool.tile([128, QT], F32, tag="ps512")
                for k in range(KH):
                    nc.tensor.matmul(pe, wh2e[:, k], cmb[:, k],
                                     start=(k == 0), stop=(k == KH - 1))
                esb = opool.tile([128, QT], BF, tag="esb")
                nc.scalar.activation(esb, pe, AF.Identity, bias=bh2e[:, 0:1])
                pl = pspool.tile([V, QT], F32, tag="ps512")
                nc.tensor.matmul(pl, emb, esb, start=True, stop=True)
                lo = opool.tile([V, QT], F32, tag="lo")
                nc.vector.tensor_copy(lo, pl)
                nc.sync.dma_start(
                    t_["logitsT"].ap().rearrange("v (t b) -> v t b", b=BL)[
                        :, ts(qi, QT), b], lo)


def _host_prep(x, context, h, embedding, W_ctx, b_ctx,
               Wi0, Wh0, bi0, bh0, Wi1, Wh1, bi1, bh1, Wi2, Wh2, bi2, bh2,
               ln_g, ln_b, Wa, W_comb, b_comb, W_h2e, b_h2e, L):
    TOK = BL * L
    f = np.float32
    x = np.asarray(x)
    # folded layer-0 weights
    WeT = (embedding.astype(f) @ Wi0[:, :E].astype(f).T)          # (72, 1536)
    Wc = (Wi0[:, E:].astype(f) @ W_ctx.astype(f))                 # (1536, 300)
    bias0 = bi0 + bh0 + Wi0[:, E:].astype(f) @ b_ctx.astype(f)
    bias1 = bi1 + bh1
    bias2 = bi2 + bh2

    def to_bf(a):
        return np.ascontiguousarray(a).astype(NPBF)

    shared = dict(
        WeT=to_bf(WeT),
        WcT=to_bf(Wc.T.reshape(3, 100, H3)),
        W1T=to_bf(Wi1.T.reshape(KH, 128, H3).transpose(1, 0, 2)),
        W2T=to_bf(Wi2.T.reshape(KH, 128, H3).transpose(1, 0, 2)),
        WhT=to_bf(np.stack([W.T.reshape(KH, 128, H3).transpose(1, 0, 2)
                            for W in (Wh0, Wh1, Wh2)])),
        biasP=np.stack([b.reshape(MJ, 128).T for b in (bias0, bias1, bias2)],
                       axis=1).astype(f),
        gamP=ln_g.reshape(KH, 128).T.astype(f),
        betP=ln_b.reshape(KH, 128).T.astype(f),
        WaT=to_bf(Wa.T.reshape(KH, 128, H).transpose(1, 0, 2)),
        WcombT=to_bf(W_comb.T.reshape(8, 128, H).transpose(1, 0, 2)),
        bcombP=b_comb.reshape(KH, 128).T.astype(f),
        Wh2eT=to_bf(W_h2e.T.reshape(KH, 128, E).transpose(1, 0, 2)),
        bh2eP=b_h2e.reshape(1, 128).T.astype(f),
        embT=to_bf(embedding.T),
        maskC=((np.arange(1024)[None, :] - 512) >= np.arange(128)[:, None]
               ).astype(NPBF),
        onesP=np.ones((128, 1), NPBF),
    )
    in_maps = []
    for c in range(NC):
        bs = slice(c * BL, (c + 1) * BL)
        xs = np.asarray(x[bs])                                   # (BL, L)
        oh = (xs[:, :, None] == np.arange(V)).astype(NPBF)       # (BL,L,V)
        ohT = np.ascontiguousarray(oh.transpose(2, 1, 0)).reshape(V, TOK)
        cs = np.asarray(context[bs], f).reshape(BL, L, 300)
        ctxT = np.ascontiguousarray(cs.transpose(2, 1, 0)).reshape(
            3, 100, TOK).astype(NPBF)
        h0 = np.asarray(h[:, bs], f)                             # (3, BL, H)
        h0P = np.ascontiguousarray(
            h0.reshape(3, BL, KH, 128).transpose(3, 0, 2, 1))
        m = dict(shared)
        m.update(ohT=ohT, ctxT=ctxT, h0P=h0P)
        in_maps.append(m)
    return in_maps


def kernel(x, context, h, embedding, W_ctx, b_ctx,
           Wi0, Wh0, bi0, bh0, Wi1, Wh1, bi1, bh1, Wi2, Wh2, bi2, bh2,
           ln_g, ln_b, Wa, W_comb, b_comb, W_h2e, b_h2e):
    L = np.asarray(x).shape[1]
    assert L == _CFG["L"], f"configure _set_mode for L={L}"
    key = "prog"
    if key not in _BUILT:
        _BUILT[key] = _build_program()
    nc = _BUILT[key]
    args = dict(x=x, context=context, h=h, embedding=embedding, W_ctx=W_ctx,
                b_ctx=b_ctx, Wi0=Wi0, Wh0=Wh0, bi0=bi0, bh0=bh0, Wi1=Wi1,
                Wh1=Wh1, bi1=bi1, bh1=bh1, Wi2=Wi2, Wh2=Wh2, bi2=bi2,
                bh2=bh2, ln_g=ln_g, ln_b=ln_b, Wa=Wa, W_comb=W_comb,
                b_comb=b_comb, W_h2e=W_h2e, b_h2e=b_h2e)
    args = {k: np.asarray(v) for k, v in args.items()}
    in_maps = _host_prep(L=L, **args)
    res = run_bass_kernel_spmd(nc, in_maps, core_ids=list(range(NC)))
    global _LAST_RES
    _LAST_RES = res.results
    logits = np.empty((B, L, V), np.float32)
    h_out = np.empty((3, B, H), np.float32)
    for c in range(NC):
        lt = res.results[c]["logitsT"].reshape(V, L, BL)
        logits[c * BL:(c + 1) * BL] = lt.transpose(2, 1, 0)
        hp = res.results[c]["houtP"]                              # (128,3,KH,BL)
        h_out[:, c * BL:(c + 1) * BL] = hp.transpose(1, 3, 2, 0).reshape(
            3, BL, H)
    return logits, h_out
